# revision 4
# baseline (speedup 1.0000x reference)
"""LINK forward (gnn message passing SpMM) on 8 TRN2 NeuronCores.

out[r, :] = W_bias + sum_{e: row[e]==r} W_weight.T[col[e], :]

Strategy (1D row-wise SpMM partitioning):
  - Core k owns output rows [k*12500, (k+1)*12500).
  - Edges bucketed to cores by row; within a core, sorted by
    (col-chunk, 128-row output tile). Col space split into 4 chunks of
    25000 so gather indices fit int16.
  - W.T is converted to bf16 on host; device gathers per-edge rows
    (gpsimd.dma_gather, 256B descriptors over 16 DMA engines).
  - Segment-sum via TensorEngine: one-hot selection matrices (built with
    iota + is_equal on DVE) matmul'd against gathered rows, accumulated
    in PSUM, then added into a SBUF-resident accumulator (bias folded
    into the first chunk pass).
  - All gathers of a chunk are emitted ahead of the consuming matmul
    loop (12-deep tile pools throttle in-flight slices), so dma_gather
    issue runs at the GpSimd engine-queue floor (~2.5us per 1024-idx
    call, 4 SWDGE queues) instead of pacing on consumers; PE runs ~98%
    busy and the kernel sits at the joint gather-issue/PE floor.
"""

import sys

sys.path.insert(0, "/opt/trn_rl_repo")

import numpy as np
import ml_dtypes

import concourse.bass as bass
import concourse.tile as tile
from concourse import bacc, mybir
from concourse.bass_utils import run_bass_kernel_spmd

P = 128
D = 128            # channels
N = 100000         # nodes
NCORE = 8
RPC = N // NCORE   # rows per core = 12500
NT = (RPC + P - 1) // P          # output tiles per core = 98
LAST_TILE_ROWS = RPC - (NT - 1) * P  # 84
NCH = 4
CHSZ = 25000       # col chunk size (int16-safe)
SLICE = 1024       # gather indices per dma_gather call (ring cap ~2048 descs)
SB = SLICE // P    # blocks per full slice = 64

LAST_EXEC_NS = None

_CACHE = {}


def _prepare(edge_index):
    """Bucket/sort/pad edges. Returns (nbt [NCH,NT] block counts,
    Lc [NCH] stream lengths, idx_arrs[core][ch] int16, m_arrs[core][ch] f32)."""
    row = np.asarray(edge_index[0], dtype=np.int64)
    col = np.asarray(edge_index[1], dtype=np.int64)
    E = row.shape[0]

    core = row // RPC
    lrow = row - core * RPC
    t = lrow >> 7
    m = lrow & 127
    ch = col // CHSZ
    lcol = col - ch * CHSZ

    gid = (core * NCH + ch) * NT + t
    order = np.argsort(gid, kind="stable")
    gid_s = gid[order]
    lcol_s = lcol[order]
    m_s = m[order]

    ngroups = NCORE * NCH * NT
    cnt = np.bincount(gid, minlength=ngroups).reshape(NCORE, NCH, NT)
    nbt = np.maximum(1, -(-cnt.max(axis=0) // P))      # [NCH, NT]
    seg_len = nbt * P
    Lc = seg_len.sum(axis=1)                            # [NCH]
    seg_start = np.zeros((NCH, NT), np.int64)
    seg_start[:, 1:] = np.cumsum(seg_len, axis=1)[:, :-1]

    starts_flat = np.zeros(ngroups, np.int64)
    flat_cnt = cnt.reshape(-1)
    starts_flat[1:] = np.cumsum(flat_cnt)[:-1]
    rank = np.arange(E, dtype=np.int64) - starts_flat[gid_s]

    core_s = gid_s // (NCH * NT)
    ch_s = (gid_s // NT) % NCH
    t_s = gid_s % NT
    dest = seg_start[ch_s, t_s] + rank

    idx_arrs = [[None] * NCH for _ in range(NCORE)]
    m_arrs = [[None] * NCH for _ in range(NCORE)]
    for c in range(NCORE):
        cm = core_s == c
        for k in range(NCH):
            mask = cm & (ch_s == k)
            ia = np.zeros(Lc[k], np.int16)
            ma = np.full(Lc[k], 200.0, np.float32)
            d = dest[mask]
            ia[d] = lcol_s[mask].astype(np.int16)
            ma[d] = m_s[mask].astype(np.float32)
            idx_arrs[c][k] = ia
            m_arrs[c][k] = ma
    return nbt, Lc, idx_arrs, m_arrs


def _build(nbt, Lc):
    nc = bacc.Bacc("TRN2", target_bir_lowering=False, num_swdge_queues=4)
    wt = nc.dram_tensor("wt", [N, D], mybir.dt.bfloat16, kind="ExternalInput")
    bias = nc.dram_tensor("bias", [P, D], mybir.dt.float32, kind="ExternalInput")
    idx_d = [
        nc.dram_tensor(f"idx{k}", [P, int(Lc[k]) // 16], mybir.dt.int16,
                       kind="ExternalInput")
        for k in range(NCH)
    ]
    m_d = [
        nc.dram_tensor(f"m{k}", [P, int(Lc[k]) // P], mybir.dt.bfloat16,
                       kind="ExternalInput")
        for k in range(NCH)
    ]
    out = nc.dram_tensor("out", [RPC, D], mybir.dt.float32, kind="ExternalOutput")

    with tile.TileContext(nc) as tc:
        with tc.tile_pool(name="const", bufs=1) as cpool, \
             tc.tile_pool(name="idx", bufs=2) as ipool, \
             tc.tile_pool(name="mval", bufs=2) as mpool, \
             tc.tile_pool(name="g", bufs=12) as gpool, \
             tc.tile_pool(name="s", bufs=12) as spool, \
             tc.tile_pool(name="psum", bufs=8, space="PSUM") as pspool:

            iota16 = cpool.tile([P, P], mybir.dt.int16)
            nc.gpsimd.iota(iota16[:], pattern=[[1, P]], base=0, channel_multiplier=0)
            iota_bf = cpool.tile([P, P], mybir.dt.bfloat16)
            nc.vector.tensor_copy(iota_bf[:], iota16[:])
            bias_t = cpool.tile([P, D], mybir.dt.float32)
            nc.sync.dma_start(bias_t[:], bias[:])
            acc = cpool.tile([P, NT * D], mybir.dt.float32)

            for k in range(NCH):
                lck = int(Lc[k])
                idx_t = ipool.tile([P, lck // 16], mybir.dt.int16, tag="idx")
                nc.sync.dma_start(idx_t[:], idx_d[k][:])
                m_t = mpool.tile([P, lck // P], mybir.dt.bfloat16, tag="mval")
                nc.sync.dma_start(m_t[:], m_d[k][:])

                g_tiles = {}
                s_tiles = {}
                gq = [0]

                def ensure(s, k=k, idx_t=idx_t, m_t=m_t, g_tiles=g_tiles,
                           s_tiles=s_tiles, lck=lck, gq=gq):
                    if s in g_tiles:
                        return
                    n = min(SLICE, lck - s * SLICE)
                    nb_s = n // P
                    g = gpool.tile([P, nb_s, D], mybir.dt.bfloat16, tag="g")
                    nc.gpsimd.dma_gather(
                        g[:],
                        wt[k * CHSZ:(k + 1) * CHSZ, :],
                        idx_t[:, s * (SLICE // 16): s * (SLICE // 16) + n // 16],
                        n, n, D, queue_num=gq[0] % 4,
                    )
                    gq[0] += 1
                    st = spool.tile([P, nb_s * P], mybir.dt.bfloat16, tag="s")
                    nc.vector.tensor_tensor(
                        out=st[:].rearrange("p (b m) -> p b m", m=P),
                        in0=m_t[:, s * SB: s * SB + nb_s].unsqueeze(2)
                            .broadcast_to([P, nb_s, P]),
                        in1=iota_bf[:].unsqueeze(1).broadcast_to([P, nb_s, P]),
                        op=mybir.AluOpType.is_equal,
                    )
                    g_tiles[s] = g
                    s_tiles[s] = st

                # issue every gather (and sel build) for this chunk upfront;
                # pool WAW deps throttle to 12 slices in flight, keeping all
                # 4 SWDGE queues busy instead of pacing issue on consumers.
                for s in range((lck + SLICE - 1) // SLICE):
                    ensure(s)

                b = 0
                for t in range(NT):
                    nb = int(nbt[k][t])
                    ps = pspool.tile([P, D], mybir.dt.float32, space="PSUM")
                    for j in range(nb):
                        s = b // SB
                        b_loc = b % SB
                        ensure(s)
                        nc.tensor.matmul(
                            out=ps[:],
                            lhsT=s_tiles[s][:, b_loc * P:(b_loc + 1) * P],
                            rhs=g_tiles[s][:, b_loc, :],
                            start=(j == 0),
                            stop=(j == nb - 1),
                        )
                        b += 1
                    acc_sl = acc[:, t * D:(t + 1) * D]
                    if k == 0:
                        nc.vector.tensor_tensor(
                            out=acc_sl, in0=ps[:], in1=bias_t[:],
                            op=mybir.AluOpType.add,
                        )
                    else:
                        nc.vector.tensor_tensor(
                            out=acc_sl, in0=acc_sl, in1=ps[:],
                            op=mybir.AluOpType.add,
                        )

            # acc[p, t, :] holds out row t*128+p
            nc.sync.dma_start(
                out[: (NT - 1) * P, :].rearrange("(t p) d -> p t d", p=P),
                acc[:].rearrange("p (t d) -> p t d", d=D)[:, : NT - 1, :],
            )
            nc.sync.dma_start(
                out[(NT - 1) * P:, :],
                acc[:LAST_TILE_ROWS, (NT - 1) * D: NT * D],
            )
    nc.compile()
    return nc


def kernel(x=None, edge_index=None, W_weight=None, W_bias=None, _trace=False):
    global LAST_EXEC_NS
    edge_index = np.asarray(edge_index)
    W_weight = np.asarray(W_weight, dtype=np.float32)
    W_bias = np.asarray(W_bias, dtype=np.float32)

    key = (edge_index.tobytes()[:4096], edge_index.shape)
    cached = _CACHE.get(key)
    if cached is None:
        nbt, Lc, idx_arrs, m_arrs = _prepare(edge_index)
        nc = _build(nbt, Lc)
        in_maps = []
        wt_bf = np.ascontiguousarray(W_weight.T).astype(ml_dtypes.bfloat16)
        bias_b = np.tile(W_bias[None, :], (P, 1)).astype(np.float32)
        for c in range(NCORE):
            im = {"wt": wt_bf, "bias": bias_b}
            for k in range(NCH):
                ia = idx_arrs[c][k]
                im[f"idx{k}"] = np.ascontiguousarray(
                    np.tile(ia.reshape(-1, 16).T, (8, 1))).astype(np.int16)
                im[f"m{k}"] = np.ascontiguousarray(
                    m_arrs[c][k].reshape(-1, P).T).astype(ml_dtypes.bfloat16)
            in_maps.append(im)
        _CACHE[key] = (nc, in_maps)
    else:
        nc, in_maps = cached

    res = run_bass_kernel_spmd(nc, in_maps, core_ids=list(range(NCORE)),
                               trace=_trace)
    LAST_EXEC_NS = res.exec_time_ns
    outp = np.concatenate([res.results[c]["out"] for c in range(NCORE)], axis=0)
    return outp.astype(np.float32)

